# revision 3
# baseline (speedup 1.0000x reference)
import sys
sys.path.insert(0, '/opt/trn_rl_repo')
import math
import numpy as np

import concourse.bass as bass
import concourse.bacc as bacc
import concourse.mybir as mybir
import concourse.tile as tile
from concourse.bass_utils import run_bass_kernel_spmd

N, E, NG = 10000, 160000, 128
HID, MID = 128, 64
NR, NS = 3, 2
CUTOFF = 8.0
PI = math.pi
NCORES = 8
P = 128
TW = 512  # edge tile width

F32 = mybir.dt.float32


def _silu(x):
    return x / (1.0 + np.exp(-x))


def _seg_argmin(vals, idx, num_seg):
    mv = np.full(num_seg, np.inf, np.float64)
    np.minimum.at(mv, idx, vals.astype(np.float64))
    cand = np.where(vals.astype(np.float64) == mv[idx], np.arange(E), E)
    a = np.full(num_seg, E, np.int64)
    np.minimum.at(a, idx, cand)
    return np.where(a >= E, 0, a)


def _get_features(dist, vecs, i, j):
    argmin0 = _seg_argmin(dist, i, N)
    n0 = j[argmin0]
    add0 = np.zeros(E, np.float32)
    add0[argmin0] = CUTOFF
    dist1 = dist + add0
    argmin1 = _seg_argmin(dist1, i, N)
    n1 = j[argmin1]
    argmin0_j = _seg_argmin(dist, j, N)
    n0_j = i[argmin0_j]
    add0j = np.zeros(E, np.float32)
    add0j[argmin0_j] = CUTOFF
    dist1_j = dist + add0j
    argmin1_j = _seg_argmin(dist1_j, j, N)

    n0e, n0je = n0[i], n0_j[j]
    idx_iref = np.where(n0e == j, argmin1[i], argmin0[i])
    idx_jref = np.where(n0je == i, argmin1_j[j], argmin0_j[j])
    pos_ji = vecs
    pos_in0 = vecs[argmin0[i]]
    pos_in1 = vecs[argmin1[i]]
    pos_iref = vecs[idx_iref]
    pos_jref_j = vecs[idx_jref]

    a = (-pos_ji * pos_in0).sum(-1)
    b = np.linalg.norm(np.cross(-pos_ji, pos_in0), axis=-1)
    theta = np.arctan2(b, a)
    theta = np.where(theta < 0, theta + PI, theta)

    p1 = np.cross(-pos_ji, pos_in0)
    p2 = np.cross(-pos_ji, pos_in1)
    a = (p1 * p2).sum(-1)
    b = (np.cross(p1, p2) * pos_ji).sum(-1) / dist
    phi = np.arctan2(b, a)
    phi = np.where(phi < 0, phi + PI, phi)

    p1 = np.cross(pos_ji, pos_jref_j)
    p2 = np.cross(pos_ji, pos_iref)
    a = (p1 * p2).sum(-1)
    b = (np.cross(p1, p2) * pos_ji).sum(-1) / dist
    tau = np.arctan2(b, a)
    tau = np.where(tau < 0, tau + PI, tau)
    return theta, phi, tau


def _rbf(d):
    n = np.arange(1, NR + 1, dtype=np.float32)
    return np.sqrt(2.0 / CUTOFF) * np.sin(n * PI * d[:, None] / CUTOFF) / d[:, None]


def _torsion_emb(d, theta, phi):
    ls = np.arange(NS, dtype=np.float32)
    ang = np.cos(ls[None, :, None] * theta[:, None, None]) * np.cos(
        ls[None, None, :] * phi[:, None, None])
    ang = ang.reshape(E, NS * NS)
    return (_rbf(d)[:, :, None] * ang[:, None, :]).reshape(E, NR * NS * NS)


def _angle_emb(d, tau):
    ls = np.arange(NS, dtype=np.float32)
    ang = np.cos(ls[None, :] * tau[:, None])
    return (_rbf(d)[:, :, None] * ang[:, None, :]).reshape(E, NR * NS)


def _build_edge_program(Ec):
    """SPMD program: per core computes f1/f2 MLPs + both convs' edge stages.

    Inputs (per core, feature-major): ebondT [128,Ec], f1inT [12,Ec],
    f2inT [6,Ec], xsrcT [128,Ec]; weights replicated.
    Outputs: m0T, m1T [128,Ec] (weighted messages for conv1/conv2).
    """
    nc = bacc.Bacc('TRN2', target_bir_lowering=False, debug=False,
                   num_devices=NCORES)
    tc = tile.TileContext(nc)
    tc.__enter__()

    ebondT = nc.dram_tensor("ebondT", [P, Ec], F32, kind="ExternalInput")
    f1inT = nc.dram_tensor("f1inT", [NR * NS * NS, Ec], F32, kind="ExternalInput")
    f2inT = nc.dram_tensor("f2inT", [NR * NS, Ec], F32, kind="ExternalInput")
    xsrcT = nc.dram_tensor("xsrcT", [P, Ec], F32, kind="ExternalInput")

    wnames = {}
    for nm, shp in [
        ("wf1_1", [12, MID]), ("bf1_1", [MID, 1]), ("wf1_2", [MID, HID]),
        ("bf1_2", [HID, 1]), ("wf2_1", [6, MID]), ("bf2_1", [MID, 1]),
        ("wf2_2", [MID, HID]), ("bf2_2", [HID, 1]),
        ("w1e_0", [P, P]), ("w1f_0", [P, P]), ("b1_0", [P, 1]),
        ("w2_0", [P, P]), ("b2_0", [P, 1]), ("wa_0", [P, 1]), ("ba_0", [1, 1]),
        ("w1e_1", [P, P]), ("w1f_1", [P, P]), ("b1_1", [P, 1]),
        ("w2_1", [P, P]), ("b2_1", [P, 1]), ("wa_1", [P, 1]), ("ba_1", [1, 1]),
        ("onesr", [1, P]),
    ]:
        wnames[nm] = nc.dram_tensor(nm, shp, F32, kind="ExternalInput")

    m0T = nc.dram_tensor("m0T", [P, Ec], F32, kind="ExternalOutput")
    m1T = nc.dram_tensor("m1T", [P, Ec], F32, kind="ExternalOutput")

    ACT = mybir.ActivationFunctionType
    NT = Ec // TW

    with (
        tc.tile_pool(name="wpool", bufs=1) as wp,
        tc.tile_pool(name="sbuf", bufs=3) as sp,
        tc.tile_pool(name="psA", bufs=1, space="PSUM") as ppA,
        tc.tile_pool(name="psB", bufs=1, space="PSUM") as ppB,
        tc.tile_pool(name="psS", bufs=1, space="PSUM") as ppS,
        tc.tile_pool(name="psF", bufs=1, space="PSUM") as ppF,
    ):
        w = {}
        for nm, t in wnames.items():
            shp = t.shape
            w[nm] = wp.tile(list(shp), F32, tag=nm, name=nm)
            nc.sync.dma_start(w[nm][:], t.ap())

        for t in range(NT):
            sl = bass.ts(t, TW)
            eb = sp.tile([P, TW], F32, tag="eb")
            nc.sync.dma_start(eb[:], ebondT.ap()[:, sl])
            xs = sp.tile([P, TW], F32, tag="xs")
            nc.sync.dma_start(xs[:], xsrcT.ap()[:, sl])
            fi1 = sp.tile([12, TW], F32, tag="fi1")
            nc.sync.dma_start(fi1[:], f1inT.ap()[:, sl])
            fi2 = sp.tile([6, TW], F32, tag="fi2")
            nc.sync.dma_start(fi2[:], f2inT.ap()[:, sl])

            fts = []
            for k, (fi, k1, b1k, k2, b2k) in enumerate([
                (fi1, "wf1_1", "bf1_1", "wf1_2", "bf1_2"),
                (fi2, "wf2_1", "bf2_1", "wf2_2", "bf2_2"),
            ]):
                pf = ppF.tile([MID, TW], F32, tag="pf")
                nc.tensor.matmul(pf[:], lhsT=w[k1][:], rhs=fi[:], start=True, stop=True)
                hmid = sp.tile([MID, TW], F32, tag=f"hmid")
                nc.scalar.activation(hmid[:], pf[:], ACT.Silu, bias=w[b1k][:])
                pf2 = ppF.tile([HID, TW], F32, tag="pf2")
                nc.tensor.matmul(pf2[:], lhsT=w[k2][:], rhs=hmid[:], start=True, stop=True)
                ft = sp.tile([HID, TW], F32, tag=f"ft{k}")
                nc.scalar.activation(ft[:], pf2[:], ACT.Silu, bias=w[b2k][:])
                fts.append(ft)

            for k, (ft, outT) in enumerate([(fts[0], m0T), (fts[1], m1T)]):
                s = str(k)
                pA = ppA.tile([P, TW], F32, tag="pA")
                nc.tensor.matmul(pA[:], lhsT=w["w1e_" + s][:], rhs=eb[:],
                                 start=True, stop=False)
                nc.tensor.matmul(pA[:], lhsT=w["w1f_" + s][:], rhs=ft[:],
                                 start=False, stop=True)
                h1 = sp.tile([P, TW], F32, tag="h1")
                nc.scalar.activation(h1[:], pA[:], ACT.Silu, bias=w["b1_" + s][:])
                pB = ppB.tile([P, TW], F32, tag="pB")
                nc.tensor.matmul(pB[:], lhsT=w["w2_" + s][:], rhs=h1[:],
                                 start=True, stop=True)
                ew = sp.tile([P, TW], F32, tag="ew")
                nc.scalar.activation(ew[:], pB[:], ACT.Identity, bias=w["b2_" + s][:])
                m = sp.tile([P, TW], F32, tag="m")
                nc.vector.tensor_mul(m[:], ew[:], xs[:])
                pS = ppS.tile([1, TW], F32, tag="pS")
                nc.tensor.matmul(pS[:], lhsT=w["wa_" + s][:], rhs=m[:],
                                 start=True, stop=True)
                sg = sp.tile([1, TW], F32, tag="sg")
                nc.scalar.activation(sg[:], pS[:], ACT.Sigmoid, bias=w["ba_" + s][:])
                pBc = ppS.tile([P, TW], F32, tag="pBc")
                nc.tensor.matmul(pBc[:], lhsT=w["onesr"][:], rhs=sg[:],
                                 start=True, stop=True)
                mo = sp.tile([P, TW], F32, tag="mo")
                nc.vector.tensor_mul(mo[:], m[:], pBc[:])
                nc.sync.dma_start(outT.ap()[:, sl], mo[:])

    tc.__exit__(None, None, None)
    nc.compile()
    return nc


def kernel(z, pos, edge_index, edge_attr, batch, params):
    z = np.asarray(z)
    pos = np.asarray(pos, np.float32)
    edge_index = np.asarray(edge_index)
    edge_attr = np.asarray(edge_attr, np.float32)
    batch = np.asarray(batch)
    pr = params

    j = edge_index[0].astype(np.int64)
    i = edge_index[1].astype(np.int64)
    vecs = pos[j] - pos[i]
    dist = np.sqrt((vecs ** 2).sum(-1)).astype(np.float32)
    theta, phi, tau = _get_features(dist, vecs, i, j)
    f1_in = _torsion_emb(dist, theta, phi).astype(np.float32)
    f2_in = _angle_emb(dist, tau).astype(np.float32)

    # dst-sorted edge partition across cores, split at node boundaries
    perm = np.argsort(i, kind="stable")
    i_s = i[perm]
    starts = [0]
    for c in range(1, NCORES):
        p0 = c * E // NCORES
        node = i_s[p0]
        starts.append(int(np.searchsorted(i_s, node, side="left")))
    starts.append(E)
    core_edges = [perm[starts[c]:starts[c + 1]] for c in range(NCORES)]
    nlo = [0] + [int(i[ce[0]]) for ce in core_edges[1:]]
    nhi = nlo[1:] + [N]
    Ec = max(len(ce) for ce in core_edges)
    Ec = ((Ec + TW - 1) // TW) * TW

    def padT(a, width):
        out = np.zeros((width, Ec), np.float32)
        out[:, :a.shape[0]] = a.T
        return out

    ebondT_c = [padT(edge_attr[ce], P) for ce in core_edges]
    f1inT_c = [padT(f1_in[ce], 12) for ce in core_edges]
    f2inT_c = [padT(f2_in[ce], 6) for ce in core_edges]
    jsrc_c = [j[ce] for ce in core_edges]
    idst_c = [i[ce] for ce in core_edges]

    nc = _build_edge_program(Ec)

    emb = np.asarray(pr["emb"], np.float32)
    x = _silu(emb[z]).astype(np.float32)

    def lin(p, v):
        y = v @ np.asarray(p["W"], np.float32).T
        if "b" in p:
            y = y + np.asarray(p["b"], np.float32)
        return y

    cnt = np.maximum(np.bincount(batch, minlength=NG), 1).astype(np.float32)[:, None]

    for bp in pr["blocks"]:
        x_pre = _silu(lin(bp["lin"], x)).astype(np.float32)

        in_maps = []
        for c in range(NCORES):
            m = {
                "ebondT": ebondT_c[c], "f1inT": f1inT_c[c], "f2inT": f2inT_c[c],
                "xsrcT": padT(x_pre[jsrc_c[c]], P),
                "onesr": np.ones((1, P), np.float32),
            }
            for k, key in enumerate(["conv1", "conv2"]):
                cp = bp[key]
                w1t = np.asarray(cp["el1"]["W"], np.float32).T  # [256,128]
                m[f"w1e_{k}"] = np.ascontiguousarray(w1t[:P])
                m[f"w1f_{k}"] = np.ascontiguousarray(w1t[P:])
                m[f"b1_{k}"] = np.asarray(cp["el1"]["b"], np.float32)[:, None]
                m[f"w2_{k}"] = np.ascontiguousarray(np.asarray(cp["el2"]["W"], np.float32).T)
                m[f"b2_{k}"] = np.asarray(cp["el2"]["b"], np.float32)[:, None]
                m[f"wa_{k}"] = np.ascontiguousarray(np.asarray(cp["attn"]["W"], np.float32).T)
                m[f"ba_{k}"] = np.asarray(cp["attn"]["b"], np.float32)[:, None]
            for mlp, w1, b1, w2, b2 in [
                (pr["lf1"], "wf1_1", "bf1_1", "wf1_2", "bf1_2"),
                (pr["lf2"], "wf2_1", "bf2_1", "wf2_2", "bf2_2"),
            ]:
                m[w1] = np.ascontiguousarray(np.asarray(mlp["l1"]["W"], np.float32).T)
                m[b1] = np.asarray(mlp["l1"]["b"], np.float32)[:, None]
                m[w2] = np.ascontiguousarray(np.asarray(mlp["l2"]["W"], np.float32).T)
                m[b2] = np.asarray(mlp["l2"]["b"], np.float32)[:, None]
            in_maps.append(m)

        res = run_bass_kernel_spmd(nc, in_maps, core_ids=list(range(NCORES)))

        hs = []
        for k, key in enumerate(["conv1", "conv2"]):
            agg = np.zeros((N, HID), np.float32)
            for c in range(NCORES):
                ne = len(core_edges[c])
                mrows = res.results[c][f"m{k}T"].T[:ne]
                ids = idst_c[c]
                uniq, st = np.unique(ids, return_index=True)
                sums = np.add.reduceat(mrows, st, axis=0)
                agg[uniq] = sums
            cp = bp[key]
            conv_out = lin(cp["lin_rel"], agg) + lin(cp["lin_root"], x_pre)
            hs.append(_silu(lin(bp[f"lin{k + 1}"], conv_out)))

        h = lin(bp["lin_cat"], np.concatenate(hs, 1)) + x_pre
        for lp in bp["lins"]:
            h = _silu(lin(lp, h)) + h

        # graph norm
        np_ = bp["norm"]
        meansum = np.zeros((NG, HID), np.float32)
        np.add.at(meansum, batch, h)
        mean = meansum / cnt
        out = h - mean[batch] * np.asarray(np_["mean_scale"], np.float32)
        varsum = np.zeros((NG, HID), np.float32)
        np.add.at(varsum, batch, out ** 2)
        var = varsum / cnt
        h = out / np.sqrt(var[batch] + 1e-5) * np.asarray(np_["weight"], np.float32) \
            + np.asarray(np_["bias"], np.float32)
        x = lin(bp["final"], h).astype(np.float32)

    return x
